# revision 1
# baseline (speedup 1.0000x reference)
"""Bahdanau self-attention kernel for Trainium2 (8 NeuronCores, Bass/Tile).

Math (per batch b):
  Wi = B @ W.T                                  [N, D]
  S[i, j]  = sum_d v[d] * tanh(Wi[i,d] + Wi[j,d])   (symmetric)
  A = softmax(S, axis=-1)
  C = A @ B

Shapes: B [4, 512, 128], W [128, 128], v [128].

Sharding: 8 cores; core c handles batch b = c // 2, query rows
q0 = (c % 2) * 256 .. q0 + 255.  Each core receives its batch's rows
ROTATED so that its 256 query rows are rows 0..255 of its local key
matrix (softmax and the attention-weighted sum are invariant to key
order).  W / v are replicated, so one SPMD program serves all cores.

Per-core pipeline (all layouts put D=128 on partitions):
  - wik[d, n] = (W @ Bk^T)[d, n] via one PE matmul (W^T and Bk^T come
    pre-transposed from the host: pure data layout, no host FLOPs).
  - for each query iq: the tanh argument is wik + wik[:, iq] broadcast
    along the free axis.  DVE tensor_scalar_add builds stacked ragged
    inputs (up to STACK queries per ACT instruction to amortize the
    ~352-cycle ACT overhead); ACT applies tanh at 1 elem/cycle/lane
    writing fp32r (tf32-like) output.
  - v-reduction over d (partitions) via PE: lhsT is a shifted view of a
    buffer holding v at column 128 and zeros elsewhere, so
    lhsT(i)[:, m] = v * (m == i); all matmuls of a block accumulate S
    rows into one [128, 512] PSUM tile (fp32r: 1 cycle/row vs 4 for
    fp32; rel err ~3e-4, gate is 2e-2).
  - symmetry: S's query square is symmetric.  Query iq only computes
    j in [iq & ~1, 512) (even-aligned for fp32r); the strict-lower part
    of the diagonal block is filled by PE-transpose-accumulating a
    masked copy, and block 1's j in [0, 128) columns are the transpose
    of block 0's S[:, 128:256].  ~24% less tanh/add/matmul work.
  - softmax: no max-subtraction needed (|S| <= sum(v) ~ 64 keeps exp in
    f32 range); ACT exp with fused free-axis accumulate (row sums),
    DVE reciprocal.
  - C: PE-transpose E, 4 accumulating matmuls against Bk, scale rows by
    the reciprocal sums, DMA out.

TimelineSim estimate: ~106 us/core; ScalarE(tanh)-bound at ~84% occupancy.
"""

import time

import numpy as np
from contextlib import ExitStack

import concourse.bacc as bacc
import concourse.mybir as mybir
import concourse.tile as tile
from concourse.bass_utils import run_bass_kernel_spmd
from concourse.masks import make_identity

F32 = mybir.dt.float32
F32R = mybir.dt.float32r
P = 128  # partitions == feature dim D
N = 512  # sequence length per batch
NB = 4  # batches
NCORES = 8
NQ = 256  # queries per core
NBLK = NQ // P  # query blocks of 128 per core
STACK = 16  # queries per ACT instruction

TRACE = False
LAST_RESULT = None  # BassKernelResults of the most recent run (for profiling)

_program = None


def _groups(ib):
    """Group schedule for block ib: list of (local_start, size).

    Sizes fill the STACK*N stack tile (more queries per ACT call as the
    ragged width shrinks), with a small ramp-in on block 0 (so the first
    ACT fires early) and a taper at the end of the last block (so the
    final PE burst before the last exp is short).  All starts/sizes even
    (fp32r matmul offsets must be even).
    """
    res = []
    q = 0
    if ib == 0:
        for s in (2, 2, 4, 8):
            res.append((q, s))
            q += s
    while q < P:
        rem = P - q
        if ib == NBLK - 1 and rem == 16:
            for s in (12, 4):
                res.append((q, s))
                q += s
            break
        c = min(rem, STACK)
        if rem > 16 and rem - c < 16:
            c = rem - 16
        res.append((q, c))
        q += c
    return res


def _build_program():
    nc = bacc.Bacc(
        "TRN2", target_bir_lowering=False, debug=False, num_devices=NCORES
    )
    Bk = nc.dram_tensor("Bk", [N, P], F32, kind="ExternalInput")
    WT = nc.dram_tensor("WT", [P, P], F32, kind="ExternalInput")
    BkT = nc.dram_tensor("BkT", [P, N], F32, kind="ExternalInput")
    vh = nc.dram_tensor("vh", [P, 4 * P], F32, kind="ExternalInput")
    # per-block masks for the diagonal-square zeroing (1 everywhere except
    # 0 on each group's diagonal square)
    mq = nc.dram_tensor("mq", [NBLK, P, P], F32, kind="ExternalInput")
    out = nc.dram_tensor("out", [NQ, P], F32, kind="ExternalOutput")

    with tile.TileContext(nc) as tc, ExitStack() as ctx:
        consts = ctx.enter_context(tc.tile_pool(name="consts", bufs=1))
        work = ctx.enter_context(tc.tile_pool(name="work", bufs=2))
        small = ctx.enter_context(tc.tile_pool(name="small", bufs=4))
        psum = ctx.enter_context(tc.tile_pool(name="psum", bufs=2, space="PSUM"))

        # preload the exp_and_others ACT table set (covers Tanh + Exp) while
        # the input DMAs are still in flight
        warm = consts.tile([P, 1], F32)
        nc.vector.memset(warm, 0.0)
        nc.scalar.activation(warm, warm, mybir.ActivationFunctionType.Tanh)

        ident = consts.tile([P, P], F32)
        make_identity(nc, ident)

        # warm the PE clock during the input-DMA window: one long fp32
        # dummy matmul (~3us at the low p-state) ramps PE to full speed
        # before the wik matmul
        zs = consts.tile([P, N], F32)
        nc.vector.memset(zs, 0.0)
        warm_ps = psum.tile([P, N], F32, tag="warmup")
        nc.tensor.matmul(warm_ps, ident, zs, start=True, stop=True)

        # critical-path DMAs first: wik matmul needs BkT (big, gating) + WT
        BkT_sb = consts.tile([P, N], F32)
        nc.sync.dma_start(out=BkT_sb, in_=BkT[:, :])
        WT_sb = consts.tile([P, P], F32)
        nc.sync.dma_start(out=WT_sb, in_=WT[:, :])
        vh_sb = consts.tile([P, 4 * P], F32)
        nc.sync.dma_start(out=vh_sb, in_=vh[:, :])
        # fp32r (tf32-like, 11-bit mantissa) rounded copy of the one-hot
        # buffer; matmuls on pre-rounded operands run at 1 cycle/row.
        # col P holds v_hi (fp32r-representable), col 3P holds v_lo.
        vhr = consts.tile([P, 4 * P], F32R)
        nc.vector.tensor_copy(vhr, vh_sb)

        mq_sb = consts.tile([P, NBLK, P], F32)
        for b_ in range(NBLK):
            nc.sync.dma_start(out=mq_sb[:, b_, :], in_=mq[b_, :, :])

        # Bk_sb[p, jb*128 + d] = Bk[jb*128 + p, d]  (key rows on partitions)
        Bk_sb = consts.tile([P, N], F32)
        for jb in range(4):
            nc.sync.dma_start(
                out=Bk_sb[:, jb * P : (jb + 1) * P], in_=Bk[jb * P : (jb + 1) * P, :]
            )

        # wik[d, n] = sum_e W[d, e] * Bk[n, e]
        wik_ps = psum.tile([P, N], F32, tag="S")
        nc.tensor.matmul(wik_ps, WT_sb, BkT_sb, start=True, stop=True)
        wik_sb = consts.tile([P, N], F32)
        nc.vector.tensor_copy(wik_sb, wik_ps)


        # Symmetry within this core's query square S[0:256, 0:256]:
        #  - a group of queries starting at gq computes j in [gq, 512)
        #    directly (group-aligned raggedness keeps fp32r matmul offsets
        #    and sizes even); the first matmul (start=True) zero-fills
        #    everything below, so uncovered cells are exact zeros
        #  - the uncovered part of the diagonal block := transpose of its
        #    copy with the per-group diagonal squares zeroed
        #  - block 1's j in [0, 128) := transpose of block 0's S[:, 128:256]
        S01_sb = None
        for ib in range(NBLK):
            groups = _groups(ib)
            jfull0 = ib * P  # diagonal block's column range start
            S_ps = psum.tile([P, N], F32, tag="S")
            for g0, gsize in groups:
                gq = ib * P + g0  # group's absolute first query
                tin = work.tile([P, STACK * N], F32, tag="tin")
                offs = []
                off = 0
                for t in range(gsize):
                    iq = gq + t
                    e = iq & ~1  # even-aligned ragged start (fp32r needs even)
                    sz = N - e
                    offs.append((off, e, sz))
                    nc.vector.tensor_scalar_add(
                        tin[:, off : off + sz],
                        wik_sb[:, e:N],
                        wik_sb[:, iq : iq + 1],
                    )
                    off += sz
                tth = work.tile([P, STACK * N], F32R, tag="tth", bufs=3)
                nc.scalar.activation(
                    tth[:, :off],
                    tin[:, :off],
                    mybir.ActivationFunctionType.Tanh,
                )
                for t in range(gsize):
                    il = g0 + t
                    toff, e, sz = offs[t]
                    # S[il, e:] += v . tanh tile via a shifted one-hot-
                    # column view of vhr (fp32r matmul: 1 cycle/row)
                    nc.tensor.matmul(
                        S_ps[:, e:N],
                        vhr[:, P - il : 2 * P - il],
                        tth[:, toff : toff + sz],
                        start=(il == 0),
                        stop=(il == P - 1),
                    )

            # mirror the uncovered lower part of the diagonal block:
            # transpose-accumulate a copy whose per-group diagonal squares
            # are zeroed (those cells were computed directly)
            Zd = work.tile([P, P], F32, tag="Zd")
            nc.vector.tensor_mul(Zd, S_ps[:, jfull0 : jfull0 + P], mq_sb[:, ib, :])
            nc.tensor.matmul(
                S_ps[:, jfull0 : jfull0 + P],
                Zd,
                ident,
                is_transpose=True,
                start=False,
                stop=True,
                skip_group_check=True,
            )

            if ib == 0:
                # stash S[0:128, 128:256] for block 1's mirrored columns
                S01_sb = work.tile([P, P], F32, tag="S01")
                nc.vector.tensor_copy(S01_sb, S_ps[:, P : 2 * P])
            else:
                # mirrored block: S[128:256, 0:128] = S01^T
                nc.tensor.transpose(S_ps[:, 0:P], S01_sb, ident)

            # no max-subtraction: |S| <= sum(v) ~ 64, exp stays in f32 range
            E_sb = work.tile([P, N], F32, tag="E")
            rsum = small.tile([P, 1], F32)
            nc.scalar.activation(
                E_sb,
                S_ps,
                mybir.ActivationFunctionType.Exp,
                accum_out=rsum,
            )
            rrec = small.tile([P, 1], F32)
            nc.vector.reciprocal(rrec, rsum)

            ET_ps = psum.tile([P, N], F32, tag="ET")
            for jb in range(4):
                nc.tensor.transpose(
                    ET_ps[:, jb * P : (jb + 1) * P], E_sb[:, jb * P : (jb + 1) * P], ident
                )
            ET_sb = work.tile([P, N], F32, tag="ET_sb")
            nc.vector.tensor_copy(ET_sb, ET_ps)

            C_ps = psum.tile([P, P], F32, tag="C")
            for jb in range(4):
                nc.tensor.matmul(
                    C_ps,
                    ET_sb[:, jb * P : (jb + 1) * P],
                    Bk_sb[:, jb * P : (jb + 1) * P],
                    start=(jb == 0),
                    stop=(jb == 3),
                )
            C_sb = work.tile([P, P], F32, tag="C_sb")
            nc.vector.tensor_scalar_mul(C_sb, C_ps, rrec)
            nc.sync.dma_start(out=out[ib * P : (ib + 1) * P, :], in_=C_sb)

    nc.compile()
    return nc


def kernel(B, W, v):
    global _program, LAST_RESULT
    B = np.ascontiguousarray(np.asarray(B, dtype=np.float32))
    W = np.ascontiguousarray(np.asarray(W, dtype=np.float32))
    v = np.asarray(v, dtype=np.float32).reshape(P)

    if _program is None:
        _program = _build_program()
    nc = _program

    # split v into fp32r-exact hi (11 mantissa bits) + lo parts
    u = v.view(np.uint32)
    v_hi = ((u + 0x800) & np.uint32(0xFFFFF000)).view(np.float32)
    v_lo = v - v_hi
    vh = np.zeros((P, 4 * P), dtype=np.float32)
    vh[:, P] = v_hi
    vh[:, 3 * P] = v_lo

    # mirror-mask for the diagonal block: keep Z[r, c] only where the
    # target cell (c, r) was NOT computed directly, i.e. r < (c & ~1)
    # (direct coverage of query iq starts at j = iq & ~1)
    r_idx = np.arange(P)[:, None]
    c_idx = np.arange(P)[None, :]
    mq = np.broadcast_to(
        (r_idx < (c_idx & ~1)).astype(np.float32), (NBLK, P, P)
    ).copy()

    WT = np.ascontiguousarray(W.T)
    in_maps = []
    for c in range(NCORES):
        b = c // 2
        q0 = (c % 2) * NQ
        Bp = np.ascontiguousarray(np.roll(B[b], -q0, axis=0))
        in_maps.append(
            {
                "Bk": Bp,
                "BkT": np.ascontiguousarray(Bp.T),
                "WT": WT,
                "vh": vh,
                "mq": mq,
            }
        )

    # retry a couple of times: the axon/PJRT execute path occasionally hits
    # transient INTERNAL errors that succeed on re-run
    res = None
    for attempt in range(3):
        try:
            res = run_bass_kernel_spmd(
                nc, in_maps, core_ids=list(range(NCORES)), trace=TRACE
            )
            break
        except Exception:
            if attempt == 2:
                raise
            time.sleep(2.0)
    LAST_RESULT = res

    C = np.empty((NB, N, P), dtype=np.float32)
    for c in range(NCORES):
        b = c // 2
        q0 = (c % 2) * NQ
        C[b, q0 : q0 + NQ] = res.results[c]["out"]
    return C



# revision 9
# speedup vs baseline: 2.3882x; 2.3882x over previous
"""Bahdanau self-attention kernel for Trainium2 (8 NeuronCores, Bass/Tile).

Math (per batch b):
  Wi = B @ W.T                                  [N, D]
  S[i, j]  = sum_d v[d] * tanh(Wi[i,d] + Wi[j,d])
  A = softmax(S, axis=-1)
  C = A @ B

Shapes: B [4, 512, 128], W [128, 128], v [128].

Sharding: 8 cores; core c handles batch b = c // 2, query rows
q0 = (c % 2) * 256 .. q0 + 255.  Each core receives its batch's rows
ROTATED so that its 256 query rows are rows 0..255 of its local key
matrix (softmax and the attention-weighted sum are invariant to key
order).

Algorithm: instead of evaluating tanh per (i, j, d) element (the
baseline; ~98k ScalarE cycles/core), expand tanh in a sine series
  tanh(x) ~= sum_F c_F sin(w_F x)
so that every term is separable across the pair:
  sin(w(a+b)) = sin(wa)cos(wb) + cos(wa)sin(wb)
and the O(N^2 D) work becomes 2 PE matmuls per frequency contracting
over d.  The ScalarE only evaluates sin/cos on the O(N D) grids.

The Sin activation is valid only on [-pi, pi], so base frequencies
satisfy w0*max|Wi| <= pi/2 (cos via bias=+pi/2 stays within [0, pi]);
higher frequencies come from exact angle-doubling chains:
  Q_{l+1} = Square(2*Q_l - 1)        (ACT, pre-affine; Q_l = cos^2(w_{l-1}))
  T_{l+1} = T_l * C_l                (DVE/Pool;  T_l = sin(w_l)/2^l)
  C_l     = 2*Q_l - 1                (DVE; = cos(w_l))
cos(w_l) = 2 Q_l - 1 is folded into matmul operands: the a-side affine
goes into tensor_scalar constants; the b-side uses raw Q_l with the
leftover term being a per-query row constant, which softmax cancels.

S is accumulated TRANSPOSED (ST[j, i]) so the exp output directly
feeds the C = A @ B matmuls without PE transposes; row sums become
tiny ones-vector matmuls.  fp32r (tf32-like) matmul operands run at
1 cycle/row; E and Bk are cast to bf16 for the output matmuls.

Fitted offline (ridge LS on tanh over [-11.14, 11.14], weighted by the
empirical |a+b| distribution): 13 frequencies from 3 doubling chains
{0.28 x L3, 0.22 x L4, 0.17 x L4} (0.56 pruned).  End-to-end numpy
emulation of this exact graph (incl. f32r/bf16 rounding): rel err
4.8e-3 vs the fp64 reference (gate: 2e-2).
"""

import time

import numpy as np
from contextlib import ExitStack

import concourse.bacc as bacc
import concourse.mybir as mybir
import concourse.tile as tile
from concourse.bass_utils import run_bass_kernel_spmd

F32 = mybir.dt.float32
F32R = mybir.dt.float32r
BF16 = mybir.dt.bfloat16
P = 128  # partitions == feature dim D
N = 512  # sequence length per batch
NB = 4  # batches
NCORES = 8
NQ = 256  # queries per core

TRACE = False
LAST_RESULT = None

_program = None

# ---- offline sine-series fit of tanh ------------------------------------
BASES = (0.28, 0.22, 0.17)
LEVELS = (3, 4, 4)  # doubling levels per chain
# (chain, level) -> coefficient; (0,1) [w=0.56] pruned from the fit
COEF = {
    (0, 0): 0.3162335487539917,
    (0, 2): 0.09987417699473285,
    (0, 3): 0.028765803452317606,
    (1, 0): 0.42793108305215616,
    (1, 1): -0.05292544499016703,
    (1, 2): 0.048513623532719284,
    (1, 3): 0.05807235903572189,
    (1, 4): 0.008246245949299827,
    (2, 0): 0.4476717378807772,
    (2, 1): 0.15458506358162089,
    (2, 2): 0.3154589666163079,
    (2, 3): 0.07077459434316662,
    (2, 4): 0.015323451391864943,
}
NCH = len(BASES)
W3 = NCH * N  # 1536: width of 3-chain batched grid tiles
W2 = 2 * N  # 1024: chains 1,2 only (level 4)

# VC scalar-column layout: for each active freq, in (chain, level) order:
#   lv0: one column  c*v
#   lv>=1: two columns  2*c*alpha*v  and  -c*alpha*v
ACTIVE = sorted(COEF.keys())


def _vc_cols():
    cols = {}
    idx = 0
    for key in ACTIVE:
        ci, lv = key
        if lv == 0:
            cols[key] = (idx,)
            idx += 1
        else:
            cols[key] = (idx, idx + 1)
            idx += 2
    return cols, idx


VC_COLS, VC_NCOL = _vc_cols()


def _build_program():
    nc = bacc.Bacc(
        "TRN2", target_bir_lowering=False, debug=False, num_devices=NCORES
    )
    BkT = nc.dram_tensor("BkT", [P, N], F32, kind="ExternalInput")
    Bk = nc.dram_tensor("Bk", [N, P], F32, kind="ExternalInput")
    WTS = nc.dram_tensor("WTS", [P, NCH * P], F32, kind="ExternalInput")
    VC = nc.dram_tensor("VC", [P, VC_NCOL], F32, kind="ExternalInput")
    out = nc.dram_tensor("out", [NQ, P], F32, kind="ExternalOutput")

    Sin = mybir.ActivationFunctionType.Sin
    Square = mybir.ActivationFunctionType.Square
    Exp = mybir.ActivationFunctionType.Exp
    MUL = mybir.AluOpType.mult
    ADD = mybir.AluOpType.add

    with tile.TileContext(nc) as tc, ExitStack() as ctx:
        consts = ctx.enter_context(tc.tile_pool(name="consts", bufs=1))
        work = ctx.enter_context(tc.tile_pool(name="work", bufs=1))
        small = ctx.enter_context(tc.tile_pool(name="small", bufs=4))
        psum = ctx.enter_context(tc.tile_pool(name="psum", bufs=1, space="PSUM"))

        # ---- phase 0: DMAs, engine warm-up --------------------------------
        # preload the ACT table set used first (Sin) while DMAs fly
        warm = consts.tile([P, 1], F32, tag="warm")
        nc.vector.memset(warm, 0.0)
        nc.scalar.activation(warm, warm, Sin)

        # warm the PE clock: one long fp32 dummy matmul during the DMA window
        zs = consts.tile([P, N], F32, tag="zs")
        nc.vector.memset(zs, 0.0)
        ident = consts.tile([P, P], F32, tag="ident")
        nc.vector.memset(ident, 0.0)
        X_ps = psum.tile([P, W3], F32, tag="X")
        nc.tensor.matmul(X_ps[:, 0:N], ident, zs, start=True, stop=True)

        BkT_sb = consts.tile([P, N], F32, tag="BkT_sb")
        nc.sync.dma_start(out=BkT_sb, in_=BkT[:, :])
        WTS_sb = consts.tile([P, NCH * P], F32, tag="WTS_sb")
        nc.sync.dma_start(out=WTS_sb, in_=WTS[:, :])
        VC_sb = consts.tile([P, VC_NCOL], F32, tag="VC_sb")
        nc.sync.dma_start(out=VC_sb, in_=VC[:, :])
        # Bk_sb[p, kb*128 + d] = Bk[kb*128 + p, d]  (key rows on partitions)
        Bk_sb = consts.tile([P, N], F32, tag="Bk_sb")
        for kb in range(4):
            nc.sync.dma_start(
                out=Bk_sb[:, kb * P : (kb + 1) * P], in_=Bk[kb * P : (kb + 1) * P, :]
            )

        # bias columns for ACT pre-affines (only 0.0/1.0 are pre-registered)
        half_pi = consts.tile([P, 1], F32, tag="half_pi")
        nc.vector.memset(half_pi, float(np.pi / 2))
        neg_one = consts.tile([P, 1], F32, tag="neg_one")
        nc.vector.memset(neg_one, -1.0)

        # fp32r copies for 1-cycle/row matmuls
        WTS_r = consts.tile([P, NCH * P], F32R, tag="WTS_r")
        nc.vector.tensor_copy(WTS_r, WTS_sb)
        BkT_r = consts.tile([P, N], F32R, tag="BkT_r")
        nc.vector.tensor_copy(BkT_r, BkT_sb)
        Bk16 = consts.tile([P, N], BF16, tag="Bk16")
        nc.vector.tensor_copy(Bk16, Bk_sb)
        ones16 = consts.tile([P, 1], BF16, tag="ones16")
        nc.vector.memset(ones16, 1.0)

        # ---- phase 1: scaled args X = w_ci * Wi^T  (PSUM, 3 banks) --------
        for ci in range(NCH):
            nc.tensor.matmul(
                X_ps[:, ci * N : (ci + 1) * N],
                WTS_r[:, ci * P : (ci + 1) * P],
                BkT_r,
                start=True,
                stop=True,
            )

        # ---- phase 2: grids ----------------------------------------------
        # base: SB = sin(X), CB = cos(X) = sin(X + pi/2)
        SB = work.tile([P, W3], F32R, tag="SB")
        nc.scalar.activation(SB, X_ps, Sin)
        CB = work.tile([P, W3], F32R, tag="CB")
        nc.scalar.activation(CB, X_ps, Sin, bias=half_pi)

        # cos chain: Q_l = cos^2(w_{l-1}); Q_{l+1} = Square(2 Q_l - 1)
        Q = {}
        Q[1] = work.tile([P, W3], F32R, tag="Q1", name="Q1")
        nc.scalar.activation(Q[1], CB, Square)
        Q[2] = work.tile([P, W3], F32R, tag="Q2", name="Q2")
        nc.scalar.activation(Q[2], Q[1], Square, scale=2.0, bias=neg_one)
        Q[3] = work.tile([P, W3], F32R, tag="Q3", name="Q3")
        nc.scalar.activation(Q[3], Q[2], Square, scale=2.0, bias=neg_one)
        Q[4] = work.tile([P, W2], F32R, tag="Q4", name="Q4")
        nc.scalar.activation(Q[4], Q[3][:, N:W3], Square, scale=2.0, bias=neg_one)

        # materialized cos grids for the sin chain: C_l = 2 Q_l - 1
        C1 = work.tile([P, W3], F32R, tag="C1")
        nc.vector.tensor_scalar(C1, Q[1], 2.0, -1.0, MUL, ADD)
        C2 = work.tile([P, W3], F32R, tag="C2")
        nc.vector.tensor_scalar(C2, Q[2], 2.0, -1.0, MUL, ADD)
        C3 = work.tile([P, W2], F32R, tag="C3")
        nc.vector.tensor_scalar(C3, Q[3][:, N:W3], 2.0, -1.0, MUL, ADD)

        # sin chain: T_l = sin(w_l)/2^l;  T_{l+1} = T_l * C_l
        T = {}
        T[1] = work.tile([P, W3], F32R, tag="T1", name="T1")
        for ci in range(NCH):
            seg = slice(ci * N, (ci + 1) * N)
            nc.vector.tensor_mul(T[1][:, seg], SB[:, seg], CB[:, seg])
        T[2] = work.tile([P, W3], F32R, tag="T2", name="T2")
        nc.gpsimd.tensor_mul(T[2], T[1], C1)
        T[3] = work.tile([P, W3], F32R, tag="T3", name="T3")
        nc.gpsimd.tensor_mul(T[3], T[2], C2)
        T[4] = work.tile([P, W2], F32R, tag="T4", name="T4")
        nc.gpsimd.tensor_mul(T[4], T[3][:, N:W3], C3)

        def sin_grid(ci, lv):
            if lv == 0:
                return SB[:, ci * N : (ci + 1) * N]
            if lv == 4:
                return T[4][:, (ci - 1) * N : ci * N]
            return T[lv][:, ci * N : (ci + 1) * N]

        def q_grid(ci, lv):
            if lv == 4:
                return Q[4][:, (ci - 1) * N : ci * N]
            return Q[lv][:, ci * N : (ci + 1) * N]

        # ---- phase 3: v-weighted a-side preps (rhs of the S^T matmuls) ----
        # a-side grid = first NQ columns of the chain segment (rotation).
        preps = {}
        for key in ACTIVE:
            ci, lv = key
            cols = VC_COLS[key]
            sg = sin_grid(ci, lv)
            pa = work.tile([P, NQ], F32R, tag=f"pa{ci}_{lv}")
            pb = work.tile([P, NQ], F32R, tag=f"pb{ci}_{lv}")
            if lv == 0:
                cv = VC_sb[:, cols[0] : cols[0] + 1]
                nc.vector.tensor_scalar_mul(pa, sg[:, :NQ], cv)
                cbs = CB[:, ci * N : ci * N + NQ]
                nc.vector.tensor_scalar_mul(pb, cbs, cv)
            else:
                c2av = VC_sb[:, cols[0] : cols[0] + 1]  # 2*c*alpha*v
                ncav = VC_sb[:, cols[1] : cols[1] + 1]  # -c*alpha*v
                nc.vector.tensor_scalar_mul(pa, sg[:, :NQ], c2av)
                qg = q_grid(ci, lv)
                nc.vector.tensor_scalar(pb, qg[:, :NQ], c2av, ncav, MUL, ADD)
            preps[key] = (pa, pb)

        # ---- phase 4: S^T accumulation ------------------------------------
        # ST[p, kb*NQ + i] = S[i, kb*128 + p]
        ST_ps = psum.tile([P, 4 * NQ], F32, tag="ST")
        order = sorted(ACTIVE, key=lambda k: k[1])  # by level: availability
        last = order[-1]
        # PSUM zero regions are whole 2KB banks: segments kb0+kb1 share a
        # bank (and kb2+kb3 the other), so each bank gets ONE accumulation
        # group: start on the first matmul touching it (zeroes the bank),
        # stop on the last.
        for key in order:
            ci, lv = key
            pa, pb = preps[key]
            sg = sin_grid(ci, lv)
            bg = CB[:, ci * N : (ci + 1) * N] if lv == 0 else q_grid(ci, lv)
            for kb in range(4):
                seg = slice(kb * NQ, (kb + 1) * NQ)
                kbs = slice(kb * P, (kb + 1) * P)
                # A-term: lhsT = cos-ish b-side block, rhs = weighted sin(a)
                nc.tensor.matmul(
                    ST_ps[:, seg],
                    bg[:, kbs],
                    pa,
                    start=(key == order[0] and kb % 2 == 0),
                    stop=False,
                )
                # B-term: lhsT = sin b-side block, rhs = weighted cos(a)
                nc.tensor.matmul(
                    ST_ps[:, seg],
                    sg[:, kbs],
                    pb,
                    start=False,
                    stop=(key == last and kb % 2 == 1),
                )

        # ---- phase 5: softmax (transposed) + C ----------------------------
        # no max-subtraction: |S| <= sum(v) ~ 62 keeps exp in f32 range
        E_sb = work.tile([P, 4 * NQ], BF16, tag="E")
        nc.scalar.activation(E_sb, ST_ps, Exp)

        # row sums rsum_i = sum_j E[j, i] via ones-matmuls; then 1/rsum
        rrec = []
        rs_ps = psum.tile([P, 2], F32, tag="rs")
        for h in range(2):
            for kb in range(4):
                nc.tensor.matmul(
                    rs_ps[:, h : h + 1],
                    E_sb[:, kb * NQ + h * P : kb * NQ + (h + 1) * P],
                    ones16,
                    start=(h == 0 and kb == 0),
                    stop=(h == 1 and kb == 3),
                )
            rr = small.tile([P, 1], F32, tag=f"rr{h}", name=f"rr{h}")
            nc.vector.reciprocal(rr, rs_ps[:, h : h + 1])
            rrec.append(rr)

        cp_ps = psum.tile([P, 2 * P], F32, tag="cp")
        for h in range(2):
            cph = cp_ps[:, h * P : (h + 1) * P]
            for kb in range(4):
                nc.tensor.matmul(
                    cph,
                    E_sb[:, kb * NQ + h * P : kb * NQ + (h + 1) * P],
                    Bk16[:, kb * P : (kb + 1) * P],
                    start=(h == 0 and kb == 0),
                    stop=(h == 1 and kb == 3),
                )
            c_sb = work.tile([P, P], F32, tag=f"c{h}", name=f"c{h}")
            nc.vector.tensor_scalar_mul(c_sb, cph, rrec[h])
            nc.sync.dma_start(out=out[h * P : (h + 1) * P, :], in_=c_sb)

    nc.compile()
    return nc


def kernel(B, W, v):
    global _program, LAST_RESULT
    B = np.ascontiguousarray(np.asarray(B, dtype=np.float32))
    W = np.ascontiguousarray(np.asarray(W, dtype=np.float32))
    v = np.asarray(v, dtype=np.float32).reshape(P)

    if _program is None:
        _program = _build_program()
    nc = _program

    WTS = np.concatenate(
        [np.float32(w0) * np.ascontiguousarray(W.T) for w0 in BASES], axis=1
    ).astype(np.float32)

    VC = np.zeros((P, VC_NCOL), dtype=np.float32)
    for key in ACTIVE:
        ci, lv = key
        cols = VC_COLS[key]
        c = COEF[key]
        if lv == 0:
            VC[:, cols[0]] = np.float32(c) * v
        else:
            alpha = float(2**lv)
            VC[:, cols[0]] = np.float32(2.0 * c * alpha) * v
            VC[:, cols[1]] = np.float32(-c * alpha) * v

    in_maps = []
    for cidx in range(NCORES):
        b = cidx // 2
        q0 = (cidx % 2) * NQ
        Bp = np.ascontiguousarray(np.roll(B[b], -q0, axis=0))
        in_maps.append(
            {
                "Bk": Bp,
                "BkT": np.ascontiguousarray(Bp.T),
                "WTS": WTS,
                "VC": VC,
            }
        )

    # retry a couple of times: the axon/PJRT execute path occasionally hits
    # transient INTERNAL errors that succeed on re-run
    res = None
    for attempt in range(3):
        try:
            res = run_bass_kernel_spmd(
                nc, in_maps, core_ids=list(range(NCORES)), trace=TRACE
            )
            break
        except Exception:
            if attempt == 2:
                raise
            time.sleep(2.0)
    LAST_RESULT = res

    C = np.empty((NB, N, P), dtype=np.float32)
    for cidx in range(NCORES):
        b = cidx // 2
        q0 = (cidx % 2) * NQ
        C[b, q0 : q0 + NQ] = res.results[cidx]["out"]
    return C


# revision 10
# speedup vs baseline: 3.3684x; 1.4105x over previous
"""Bahdanau self-attention kernel for Trainium2 (8 NeuronCores, Bass/Tile).

Math (per batch b):
  Wi = B @ W.T                                  [N, D]
  S[i, j]  = sum_d v[d] * tanh(Wi[i,d] + Wi[j,d])
  A = softmax(S, axis=-1)
  C = A @ B

Shapes: B [4, 512, 128], W [128, 128], v [128].

Sharding: 8 cores; core c handles batch b = c // 2, query rows
q0 = (c % 2) * 256 .. q0 + 255.  Each core receives its batch's rows
ROTATED so that its 256 query rows are rows 0..255 of its local key
matrix (softmax and the attention-weighted sum are invariant to key
order).

Algorithm: instead of evaluating tanh per (i, j, d) element (the
baseline; ~98k ScalarE cycles/core), expand tanh in a sine series
  tanh(x) ~= sum_F c_F sin(w_F x)
so that every term is separable across the pair:
  sin(w(a+b)) = sin(wa)cos(wb) + cos(wa)sin(wb)
and the O(N^2 D) work becomes 2 PE matmuls per frequency contracting
over d.  The ScalarE only evaluates sin/cos on the O(N D) grids.

The Sin activation is valid only on [-pi, pi], so base frequencies
satisfy w0*max|Wi| <= pi/2 (cos via bias=+pi/2 stays within [0, pi]);
higher frequencies come from exact angle-doubling chains:
  Q_{l+1} = Square(2*Q_l - 1)        (ACT, pre-affine; Q_l = cos^2(w_{l-1}))
  T_{l+1} = T_l * C_l                (DVE/Pool;  T_l = sin(w_l)/2^l)
  C_l     = 2*Q_l - 1                (DVE; = cos(w_l))
cos(w_l) = 2 Q_l - 1 is folded into matmul operands: the a-side affine
goes into tensor_scalar constants; the b-side uses raw Q_l with the
leftover term being a per-query row constant, which softmax cancels.

S is accumulated TRANSPOSED (ST[j, i]) so the exp output directly
feeds the C = A @ B matmuls without PE transposes; row sums become
tiny ones-vector matmuls.  fp32r (tf32-like) matmul operands run at
1 cycle/row; E and Bk are cast to bf16 for the output matmuls.
PSUM zero regions are whole 2KB banks, so each ST bank carries one
accumulation group (start on first touch, stop on last).

Scheduling: inputs arrive as two consolidated DMAs on separate queue
sequencers; engine queues are emitted in dependency-depth order (each
engine executes in-order); filler matmuls keep the PE p-state ramped
across the grid phase; the exp table-set load is triggered by a dummy
exp while the S matmuls still run; exp is split per ST bank so the
first half overlaps the last matmuls.

Fitted offline (ridge LS on tanh over [-11.14, 11.14], weighted by the
empirical |a+b| distribution): 13 frequencies from 3 doubling chains
{0.28 x L3, 0.22 x L4, 0.17 x L4} (0.56 pruned).  End-to-end numpy
emulation of this exact graph (incl. f32r/bf16 rounding): rel err
~5e-3 vs the fp64 reference (gate: 2e-2).
"""

import time

import numpy as np
from contextlib import ExitStack

import concourse.bacc as bacc
import concourse.mybir as mybir
import concourse.tile as tile
from concourse.bass_utils import run_bass_kernel_spmd

F32 = mybir.dt.float32
F32R = mybir.dt.float32r
BF16 = mybir.dt.bfloat16
P = 128  # partitions == feature dim D
N = 512  # sequence length per batch
NB = 4  # batches
NCORES = 8
NQ = 256  # queries per core

TRACE = False
LAST_RESULT = None

_program = None

# ---- offline sine-series fit of tanh ------------------------------------
BASES = (0.28, 0.22, 0.17)
LEVELS = (3, 4, 4)  # doubling levels per chain
# (chain, level) -> coefficient; (0,1) [w=0.56] pruned from the fit
COEF = {
    (0, 0): 0.3162335487539917,
    (0, 2): 0.09987417699473285,
    (0, 3): 0.028765803452317606,
    (1, 0): 0.42793108305215616,
    (1, 1): -0.05292544499016703,
    (1, 2): 0.048513623532719284,
    (1, 3): 0.05807235903572189,
    (1, 4): 0.008246245949299827,
    (2, 0): 0.4476717378807772,
    (2, 1): 0.15458506358162089,
    (2, 2): 0.3154589666163079,
    (2, 3): 0.07077459434316662,
    (2, 4): 0.015323451391864943,
}
NCH = len(BASES)
W3 = NCH * N  # 1536: width of 3-chain batched grid tiles
W2 = 2 * N  # 1024: chains 1,2 only (level 4)

ACTIVE = sorted(COEF.keys())
N_FILLERS = 8  # PE clock-keepalive matmuls between X and the S phase

# consolidated input 1 layout: [BkT (512) | WTS (384) | VC (NCOL)]
OFF_WTS = N
OFF_VC = N + NCH * P


def _vc_cols():
    cols = {}
    idx = 0
    for key in ACTIVE:
        ci, lv = key
        if lv == 0:
            cols[key] = (idx,)
            idx += 1
        else:
            cols[key] = (idx, idx + 1)
            idx += 2
    return cols, idx


VC_COLS, VC_NCOL = _vc_cols()
W_IN1 = OFF_VC + VC_NCOL


def _build_program():
    nc = bacc.Bacc(
        "TRN2", target_bir_lowering=False, debug=False, num_devices=NCORES
    )
    IN1 = nc.dram_tensor("IN1", [P, W_IN1], F32, kind="ExternalInput")
    IN2 = nc.dram_tensor("IN2", [P, N], F32, kind="ExternalInput")
    out = nc.dram_tensor("out", [NQ, P], F32, kind="ExternalOutput")

    Sin = mybir.ActivationFunctionType.Sin
    Square = mybir.ActivationFunctionType.Square
    Exp = mybir.ActivationFunctionType.Exp
    MUL = mybir.AluOpType.mult
    ADD = mybir.AluOpType.add

    with tile.TileContext(nc) as tc, ExitStack() as ctx:
        consts = ctx.enter_context(tc.tile_pool(name="consts", bufs=1))
        work = ctx.enter_context(tc.tile_pool(name="work", bufs=1))
        small = ctx.enter_context(tc.tile_pool(name="small", bufs=4))
        psum = ctx.enter_context(tc.tile_pool(name="psum", bufs=1, space="PSUM"))

        # ---- phase 0: DMAs (two queues), constants, warm-up ---------------
        IN1_sb = consts.tile([P, W_IN1], F32, tag="IN1")
        nc.sync.dma_start(out=IN1_sb, in_=IN1[:, :])
        IN2_sb = consts.tile([P, N], F32, tag="IN2")
        nc.gpsimd.dma_start(out=IN2_sb, in_=IN2[:, :])

        # DVE constants (emitted first: no dependencies)
        zs = consts.tile([P, NQ], F32, tag="zs")
        nc.vector.memset(zs, 0.0)
        half_pi = consts.tile([P, 1], F32, tag="half_pi")
        nc.vector.memset(half_pi, float(np.pi / 2))
        neg_one = consts.tile([P, 1], F32, tag="neg_one")
        nc.vector.memset(neg_one, -1.0)
        ones16 = consts.tile([P, 1], BF16, tag="ones16")
        nc.vector.memset(ones16, 1.0)

        # preload the trig ACT table set while DMAs fly
        warm = consts.tile([P, 1], F32, tag="warm")
        nc.vector.memset(warm, 0.0)
        nc.scalar.activation(warm, warm, Sin)

        # PE p-state ramp: short fp32 dummy during the DMA window
        scr_ps = psum.tile([P, N], F32, tag="scr")
        nc.tensor.matmul(scr_ps[:, :NQ], zs[:, :P], zs, start=True, stop=True)

        # f32r copy of [BkT | WTS] for 1-cycle/row matmuls
        INr = consts.tile([P, OFF_VC], F32R, tag="INr")
        nc.vector.tensor_copy(INr, IN1_sb[:, :OFF_VC])
        BkT_r = INr[:, 0:N]
        WTS_r = INr[:, OFF_WTS : OFF_WTS + NCH * P]
        VC_sb = IN1_sb[:, OFF_VC : OFF_VC + VC_NCOL]
        Bk16 = consts.tile([P, N], BF16, tag="Bk16")
        nc.vector.tensor_copy(Bk16, IN2_sb)

        # ---- phase 1: scaled args X = w_ci * Wi^T  (PSUM, 3 banks) --------
        X_ps = psum.tile([P, W3], F32, tag="X")
        for ci in range(NCH):
            nc.tensor.matmul(
                X_ps[:, ci * N : (ci + 1) * N],
                WTS_r[:, ci * P : (ci + 1) * P],
                BkT_r,
                start=True,
                stop=True,
            )
        # clock-keepalive fillers (PE executes in-order; these absorb the
        # wait for the first grids so the S matmuls start at full speed)
        for f in range(N_FILLERS):
            nc.tensor.matmul(
                scr_ps[:, :NQ],
                WTS_r[:, 0:P],
                BkT_r[:, 0:NQ],
                start=True,
                stop=True,
                skip_group_check=True,
            )

        # ---- phase 2: grids (ACT chain is the program's spine) ------------
        SB = work.tile([P, W3], F32R, tag="SB")
        nc.scalar.activation(SB, X_ps, Sin)
        CB = work.tile([P, W3], F32R, tag="CB")
        nc.scalar.activation(CB, X_ps, Sin, bias=half_pi)

        Q = {}
        Q[1] = work.tile([P, W3], F32R, tag="Q1", name="Q1")
        nc.scalar.activation(Q[1], CB, Square)
        Q[2] = work.tile([P, W3], F32R, tag="Q2", name="Q2")
        nc.scalar.activation(Q[2], Q[1], Square, scale=2.0, bias=neg_one)
        Q[3] = work.tile([P, W3], F32R, tag="Q3", name="Q3")
        nc.scalar.activation(Q[3], Q[2], Square, scale=2.0, bias=neg_one)
        Q[4] = work.tile([P, W2], F32R, tag="Q4", name="Q4")
        nc.scalar.activation(Q[4], Q[3][:, N:W3], Square, scale=2.0, bias=neg_one)
        # trigger the exp table-set load now; the real exps then pay nothing
        warm2 = small.tile([P, 1], F32, tag="warm2")
        nc.scalar.activation(warm2, warm, Exp)

        # sin chain (T_l = sin(w_l)/2^l) and cos grids (C_l = 2 Q_l - 1):
        # T on DVE/Pool, C on DVE, interleaved with the a-side preps below.
        T = {}
        C_ = {}
        T[1] = work.tile([P, W3], F32R, tag="T1", name="T1")
        for ci in range(NCH):
            seg = slice(ci * N, (ci + 1) * N)
            nc.vector.tensor_mul(T[1][:, seg], SB[:, seg], CB[:, seg])

        def sin_grid(ci, lv):
            if lv == 0:
                return SB[:, ci * N : (ci + 1) * N]
            if lv == 4:
                return T[4][:, (ci - 1) * N : ci * N]
            return T[lv][:, ci * N : (ci + 1) * N]

        def q_grid(ci, lv):
            if lv == 4:
                return Q[4][:, (ci - 1) * N : ci * N]
            return Q[lv][:, ci * N : (ci + 1) * N]

        preps = {}

        def emit_preps(lv_want):
            for key in ACTIVE:
                ci, lv = key
                if lv != lv_want:
                    continue
                cols = VC_COLS[key]
                sg = sin_grid(ci, lv)
                pa = work.tile([P, NQ], F32R, tag=f"pa{ci}_{lv}", name=f"pa{ci}_{lv}")
                pb = work.tile([P, NQ], F32R, tag=f"pb{ci}_{lv}", name=f"pb{ci}_{lv}")
                if lv == 0:
                    cv = VC_sb[:, cols[0] : cols[0] + 1]
                    nc.vector.tensor_scalar_mul(pa, sg[:, :NQ], cv)
                    cbs = CB[:, ci * N : ci * N + NQ]
                    nc.vector.tensor_scalar_mul(pb, cbs, cv)
                else:
                    c2av = VC_sb[:, cols[0] : cols[0] + 1]  # 2*c*alpha*v
                    ncav = VC_sb[:, cols[1] : cols[1] + 1]  # -c*alpha*v
                    nc.vector.tensor_scalar_mul(pa, sg[:, :NQ], c2av)
                    qg = q_grid(ci, lv)
                    nc.vector.tensor_scalar(pb, qg[:, :NQ], c2av, ncav, MUL, ADD)
                preps[key] = (pa, pb)

        emit_preps(0)  # needs SB/CB only: unblocks the first S matmuls

        C_[1] = work.tile([P, W3], F32R, tag="C1", name="C1")
        nc.vector.tensor_scalar(C_[1], Q[1], 2.0, -1.0, MUL, ADD)
        emit_preps(1)  # needs Q1/T1

        T[2] = work.tile([P, W3], F32R, tag="T2", name="T2")
        nc.gpsimd.tensor_mul(T[2], T[1], C_[1])
        C_[2] = work.tile([P, W3], F32R, tag="C2", name="C2")
        nc.vector.tensor_scalar(C_[2], Q[2], 2.0, -1.0, MUL, ADD)
        emit_preps(2)  # needs Q2/T2

        T[3] = work.tile([P, W3], F32R, tag="T3", name="T3")
        nc.gpsimd.tensor_mul(T[3], T[2], C_[2])
        C_[3] = work.tile([P, W2], F32R, tag="C3", name="C3")
        nc.vector.tensor_scalar(C_[3], Q[3][:, N:W3], 2.0, -1.0, MUL, ADD)
        emit_preps(3)  # needs Q3/T3

        T[4] = work.tile([P, W2], F32R, tag="T4", name="T4")
        nc.gpsimd.tensor_mul(T[4], T[3][:, N:W3], C_[3])
        emit_preps(4)  # needs Q4/T4

        # ---- phase 3: S^T accumulation ------------------------------------
        # ST[p, kb*NQ + i] = S[i, kb*128 + p].  Banks: {kb0,kb1} and
        # {kb2,kb3}; one accumulation group per bank.
        ST_ps = psum.tile([P, 4 * NQ], F32, tag="ST")
        order = sorted(ACTIVE, key=lambda k: k[1])
        last = order[-1]
        for key in order:
            ci, lv = key
            pa, pb = preps[key]
            sg = sin_grid(ci, lv)
            bg = CB[:, ci * N : (ci + 1) * N] if lv == 0 else q_grid(ci, lv)
            for kb in range(4):
                seg = slice(kb * NQ, (kb + 1) * NQ)
                kbs = slice(kb * P, (kb + 1) * P)
                # A-term: lhsT = cos-ish b-side block, rhs = weighted sin(a)
                nc.tensor.matmul(
                    ST_ps[:, seg],
                    bg[:, kbs],
                    pa,
                    start=(key == order[0] and kb % 2 == 0),
                    stop=False,
                )
                # B-term: lhsT = sin b-side block, rhs = weighted cos(a)
                nc.tensor.matmul(
                    ST_ps[:, seg],
                    sg[:, kbs],
                    pb,
                    start=False,
                    stop=(key == last and kb % 2 == 1),
                )

        # ---- phase 4: softmax (transposed) + C ----------------------------
        # no max-subtraction: |S| <= sum(v) ~ 62 keeps exp in f32 range.
        # One exp per ST bank so the first overlaps the last matmuls.
        E_sb = work.tile([P, 4 * NQ], BF16, tag="E")
        nc.scalar.activation(E_sb[:, 0:N], ST_ps[:, 0:N], Exp)
        nc.scalar.activation(E_sb[:, N : 2 * N], ST_ps[:, N : 2 * N], Exp)

        # row sums rsum_i = sum_j E[j, i] via ones-matmuls; then 1/rsum
        rrec = []
        rs_ps = psum.tile([P, 2], F32, tag="rs")
        for h in range(2):
            for kb in range(4):
                nc.tensor.matmul(
                    rs_ps[:, h : h + 1],
                    E_sb[:, kb * NQ + h * P : kb * NQ + (h + 1) * P],
                    ones16,
                    start=(h == 0 and kb == 0),
                    stop=(h == 1 and kb == 3),
                )
            rr = small.tile([P, 1], F32, tag=f"rr{h}", name=f"rr{h}")
            nc.vector.reciprocal(rr, rs_ps[:, h : h + 1])
            rrec.append(rr)

        cp_ps = psum.tile([P, 2 * P], F32, tag="cp")
        for h in range(2):
            cph = cp_ps[:, h * P : (h + 1) * P]
            for kb in range(4):
                nc.tensor.matmul(
                    cph,
                    E_sb[:, kb * NQ + h * P : kb * NQ + (h + 1) * P],
                    Bk16[:, kb * P : (kb + 1) * P],
                    start=(h == 0 and kb == 0),
                    stop=(h == 1 and kb == 3),
                )
            c_sb = work.tile([P, P], F32, tag=f"c{h}", name=f"c{h}")
            nc.vector.tensor_scalar_mul(c_sb, cph, rrec[h])
            nc.sync.dma_start(out=out[h * P : (h + 1) * P, :], in_=c_sb)

    nc.compile()
    return nc


def kernel(B, W, v):
    global _program, LAST_RESULT
    B = np.ascontiguousarray(np.asarray(B, dtype=np.float32))
    W = np.ascontiguousarray(np.asarray(W, dtype=np.float32))
    v = np.asarray(v, dtype=np.float32).reshape(P)

    if _program is None:
        _program = _build_program()
    nc = _program

    WTS = np.concatenate(
        [np.float32(w0) * np.ascontiguousarray(W.T) for w0 in BASES], axis=1
    ).astype(np.float32)

    VC = np.zeros((P, VC_NCOL), dtype=np.float32)
    for key in ACTIVE:
        ci, lv = key
        cols = VC_COLS[key]
        c = COEF[key]
        if lv == 0:
            VC[:, cols[0]] = np.float32(c) * v
        else:
            alpha = float(2**lv)
            VC[:, cols[0]] = np.float32(2.0 * c * alpha) * v
            VC[:, cols[1]] = np.float32(-c * alpha) * v

    in_maps = []
    for cidx in range(NCORES):
        b = cidx // 2
        q0 = (cidx % 2) * NQ
        Bp = np.ascontiguousarray(np.roll(B[b], -q0, axis=0))
        in1 = np.concatenate([Bp.T, WTS, VC], axis=1).astype(np.float32)
        # IN2[p, kb*128 + d] = Bp[kb*128 + p, d]  (key rows on partitions)
        in2 = np.ascontiguousarray(
            Bp.reshape(4, P, P).transpose(1, 0, 2).reshape(P, N)
        )
        in_maps.append({"IN1": np.ascontiguousarray(in1), "IN2": in2})

    # retry a couple of times: the axon/PJRT execute path occasionally hits
    # transient INTERNAL errors that succeed on re-run
    res = None
    for attempt in range(3):
        try:
            res = run_bass_kernel_spmd(
                nc, in_maps, core_ids=list(range(NCORES)), trace=TRACE
            )
            break
        except Exception:
            if attempt == 2:
                raise
            time.sleep(2.0)
    LAST_RESULT = res

    C = np.empty((NB, N, P), dtype=np.float32)
    for cidx in range(NCORES):
        b = cidx // 2
        q0 = (cidx % 2) * NQ
        C[b, q0 : q0 + NQ] = res.results[cidx]["out"]
    return C


# revision 12
# speedup vs baseline: 3.5720x; 1.0604x over previous
"""Bahdanau self-attention kernel for Trainium2 (8 NeuronCores, Bass/Tile).

Math (per batch b):
  Wi = B @ W.T                                  [N, D]
  S[i, j]  = sum_d v[d] * tanh(Wi[i,d] + Wi[j,d])
  A = softmax(S, axis=-1)
  C = A @ B

Shapes: B [4, 512, 128], W [128, 128], v [128].

Sharding: 8 cores; core c handles batch b = c // 2, query rows
q0 = (c % 2) * 256 .. q0 + 255.  Each core receives its batch's rows
ROTATED so that its 256 query rows are rows 0..255 of its local key
matrix (softmax and the attention-weighted sum are invariant to key
order).

Algorithm: instead of evaluating tanh per (i, j, d) element (the
baseline; ~98k ScalarE cycles/core), expand tanh in a sine series
  tanh(x) ~= sum_F c_F sin(w_F x)
so that every term is separable across the pair:
  sin(w(a+b)) = sin(wa)cos(wb) + cos(wa)sin(wb)
and the O(N^2 D) work becomes 2 PE matmuls per frequency contracting
over d.  The ScalarE only evaluates sin/cos on the O(N D) grids.

The Sin activation is valid only on [-pi, pi], so base frequencies
satisfy w0*max|Wi| <= pi/2 (cos via bias=+pi/2 stays within [0, pi]);
higher frequencies come from exact angle-doubling chains:
  Q_{l+1} = Square(2*Q_l - 1)        (ACT, pre-affine; Q_l = cos^2(w_{l-1}))
  T_{l+1} = T_l * C_l                (DVE/Pool;  T_l = sin(w_l)/2^l)
  C_l     = 2*Q_l - 1                (DVE; = cos(w_l))
cos(w_l) = 2 Q_l - 1 is folded into matmul operands: the a-side affine
goes into tensor_scalar constants; the b-side uses raw Q_l with the
leftover term being a per-query row constant, which softmax cancels.

S is accumulated TRANSPOSED (ST[j, i]) so the exp output directly
feeds the C = A @ B matmuls without PE transposes; row sums become
tiny ones-vector matmuls.  fp32r (tf32-like) matmul operands run at
1 cycle/row; E and Bk are cast to bf16 for the output matmuls.
PSUM zero regions are whole 2KB banks, so each ST bank carries one
accumulation group (start on first touch, stop on last).

Scheduling: inputs arrive as two consolidated DMAs on separate queue
sequencers; engine queues are emitted in dependency-depth order (each
engine executes in-order); filler matmuls keep the PE p-state ramped
across the grid phase; the exp table-set load is triggered by a dummy
exp while the S matmuls still run; exp is split per ST bank so the
first half overlaps the last matmuls.

Fitted offline (ridge LS on tanh over [-11.14, 11.14], weighted by the
empirical |a+b| distribution): 13 frequencies from 3 doubling chains
{0.28 x L3, 0.22 x L4, 0.17 x L4} (0.56 pruned).  End-to-end numpy
emulation of this exact graph (incl. f32r/bf16 rounding): rel err
~5e-3 vs the fp64 reference (gate: 2e-2).
"""

import time

import numpy as np
from contextlib import ExitStack

import concourse.bacc as bacc
import concourse.mybir as mybir
import concourse.tile as tile
from concourse.bass_utils import run_bass_kernel_spmd

F32 = mybir.dt.float32
F32R = mybir.dt.float32r
BF16 = mybir.dt.bfloat16
P = 128  # partitions == feature dim D
N = 512  # sequence length per batch
NB = 4  # batches
NCORES = 8
NQ = 256  # queries per core

TRACE = False
LAST_RESULT = None

_program = None

# ---- offline sine-series fit of tanh ------------------------------------
BASES = (0.28, 0.22, 0.17)
LEVELS = (3, 4, 4)  # doubling levels per chain
# (chain, level) -> coefficient; (0,1) [w=0.56] pruned from the fit
COEF = {
    (0, 0): 0.3162335487539917,
    (0, 2): 0.09987417699473285,
    (0, 3): 0.028765803452317606,
    (1, 0): 0.42793108305215616,
    (1, 1): -0.05292544499016703,
    (1, 2): 0.048513623532719284,
    (1, 3): 0.05807235903572189,
    (1, 4): 0.008246245949299827,
    (2, 0): 0.4476717378807772,
    (2, 1): 0.15458506358162089,
    (2, 2): 0.3154589666163079,
    (2, 3): 0.07077459434316662,
    (2, 4): 0.015323451391864943,
}
NCH = len(BASES)
W3 = NCH * N  # 1536: width of 3-chain batched grid tiles
W2 = 2 * N  # 1024: chains 1,2 only (level 4)

ACTIVE = sorted(COEF.keys())
N_FILLERS = 8  # PE clock-keepalive matmuls between X and the S phase

# consolidated input 1 layout: [BkT (512) | WTS (384) | VC (NCOL)]
OFF_WTS = N
OFF_VC = N + NCH * P


def _vc_cols():
    cols = {}
    idx = 0
    for key in ACTIVE:
        ci, lv = key
        if lv == 0:
            cols[key] = (idx,)
            idx += 1
        else:
            cols[key] = (idx, idx + 1)
            idx += 2
    return cols, idx


VC_COLS, VC_NCOL = _vc_cols()
W_IN1 = OFF_VC + VC_NCOL


def _build_program():
    nc = bacc.Bacc(
        "TRN2", target_bir_lowering=False, debug=False, num_devices=NCORES
    )
    BkTd = nc.dram_tensor("BkTd", [P, N], F32R, kind="ExternalInput")
    WVd = nc.dram_tensor("WVd", [P, NCH * P + VC_NCOL], F32R, kind="ExternalInput")
    IN2 = nc.dram_tensor("IN2", [P, N], F32, kind="ExternalInput")
    out = nc.dram_tensor("out", [NQ, P], F32, kind="ExternalOutput")

    Sin = mybir.ActivationFunctionType.Sin
    Square = mybir.ActivationFunctionType.Square
    Exp = mybir.ActivationFunctionType.Exp
    MUL = mybir.AluOpType.mult
    ADD = mybir.AluOpType.add

    with tile.TileContext(nc) as tc, ExitStack() as ctx:
        consts = ctx.enter_context(tc.tile_pool(name="consts", bufs=1))
        work = ctx.enter_context(tc.tile_pool(name="work", bufs=1))
        small = ctx.enter_context(tc.tile_pool(name="small", bufs=4))
        psum = ctx.enter_context(tc.tile_pool(name="psum", bufs=1, space="PSUM"))

        # ---- phase 0: DMAs (two queues), constants, warm-up ---------------
        BkT_r = consts.tile([P, N], F32R, tag="BkT_r")
        nc.sync.dma_start(out=BkT_r, in_=BkTd[:, :])
        WV_sb = consts.tile([P, NCH * P + VC_NCOL], F32R, tag="WV")
        nc.gpsimd.dma_start(out=WV_sb, in_=WVd[:, :])
        IN2_sb = consts.tile([P, N], F32, tag="IN2")
        nc.gpsimd.dma_start(out=IN2_sb, in_=IN2[:, :])

        # DVE constants (emitted first: no dependencies)
        zs = consts.tile([P, NQ], F32, tag="zs")
        nc.vector.memset(zs, 0.0)
        half_pi = consts.tile([P, 1], F32, tag="half_pi")
        nc.vector.memset(half_pi, float(np.pi / 2))
        neg_one = consts.tile([P, 1], F32, tag="neg_one")
        nc.vector.memset(neg_one, -1.0)
        ones16 = consts.tile([P, 1], BF16, tag="ones16")
        nc.vector.memset(ones16, 1.0)

        # preload the trig ACT table set while DMAs fly
        warm = consts.tile([P, 1], F32, tag="warm")
        nc.vector.memset(warm, 0.0)
        nc.scalar.activation(warm, warm, Sin)

        # PE p-state ramp: short fp32 dummy during the DMA window
        scr_ps = psum.tile([P, N], F32, tag="scr")
        nc.tensor.matmul(scr_ps[:, :NQ], zs[:, :P], zs, start=True, stop=True)

        WTS_r = WV_sb[:, 0 : NCH * P]
        VC_sb = consts.tile([P, VC_NCOL], F32, tag="VC_sb")
        nc.vector.tensor_copy(VC_sb, WV_sb[:, NCH * P : NCH * P + VC_NCOL])
        Bk16 = consts.tile([P, N], BF16, tag="Bk16")
        nc.vector.tensor_copy(Bk16, IN2_sb)

        # ---- phase 1: scaled args X = w_ci * Wi^T  (PSUM, 3 banks) --------
        X_ps = psum.tile([P, W3], F32, tag="X")
        for ci in range(NCH):
            nc.tensor.matmul(
                X_ps[:, ci * N : (ci + 1) * N],
                WTS_r[:, ci * P : (ci + 1) * P],
                BkT_r,
                start=True,
                stop=True,
            )
        # clock-keepalive fillers (PE executes in-order; these absorb the
        # wait for the first grids so the S matmuls start at full speed)
        for f in range(N_FILLERS):
            nc.tensor.matmul(
                scr_ps[:, :NQ],
                WTS_r[:, 0:P],
                BkT_r[:, 0:NQ],
                start=True,
                stop=True,
                skip_group_check=True,
            )

        # ---- phase 2: grids (ACT chain is the program's spine) ------------
        SB = work.tile([P, W3], F32R, tag="SB")
        nc.scalar.activation(SB, X_ps, Sin)
        CB = work.tile([P, W3], F32R, tag="CB")
        nc.scalar.activation(CB, X_ps, Sin, bias=half_pi)

        Q = {}
        Q[1] = work.tile([P, W3], F32R, tag="Q1", name="Q1")
        nc.scalar.activation(Q[1], CB, Square)
        Q[2] = work.tile([P, W3], F32R, tag="Q2", name="Q2")
        nc.scalar.activation(Q[2], Q[1], Square, scale=2.0, bias=neg_one)
        Q[3] = work.tile([P, W3], F32R, tag="Q3", name="Q3")
        nc.scalar.activation(Q[3], Q[2], Square, scale=2.0, bias=neg_one)
        Q[4] = work.tile([P, W2], F32R, tag="Q4", name="Q4")
        nc.scalar.activation(Q[4], Q[3][:, N:W3], Square, scale=2.0, bias=neg_one)
        # trigger the exp table-set load now; the real exps then pay nothing
        warm2 = small.tile([P, 1], F32, tag="warm2")
        nc.scalar.activation(warm2, warm, Exp)

        # sin chain (T_l = sin(w_l)/2^l) and cos grids (C_l = 2 Q_l - 1):
        # T on DVE/Pool, C on DVE, interleaved with the a-side preps below.
        T = {}
        C_ = {}
        T[1] = work.tile([P, W3], F32R, tag="T1", name="T1")
        for ci in range(NCH):
            seg = slice(ci * N, (ci + 1) * N)
            nc.vector.tensor_mul(T[1][:, seg], SB[:, seg], CB[:, seg])

        def sin_grid(ci, lv):
            if lv == 0:
                return SB[:, ci * N : (ci + 1) * N]
            if lv == 4:
                return T[4][:, (ci - 1) * N : ci * N]
            return T[lv][:, ci * N : (ci + 1) * N]

        def q_grid(ci, lv):
            if lv == 4:
                return Q[4][:, (ci - 1) * N : ci * N]
            return Q[lv][:, ci * N : (ci + 1) * N]

        preps = {}

        def emit_preps(lv_want):
            for key in ACTIVE:
                ci, lv = key
                if lv != lv_want:
                    continue
                cols = VC_COLS[key]
                sg = sin_grid(ci, lv)
                pa = work.tile([P, NQ], F32R, tag=f"pa{ci}_{lv}", name=f"pa{ci}_{lv}")
                pb = work.tile([P, NQ], F32R, tag=f"pb{ci}_{lv}", name=f"pb{ci}_{lv}")
                if lv == 0:
                    cv = VC_sb[:, cols[0] : cols[0] + 1]
                    nc.vector.tensor_scalar_mul(pa, sg[:, :NQ], cv)
                    cbs = CB[:, ci * N : ci * N + NQ]
                    nc.vector.tensor_scalar_mul(pb, cbs, cv)
                else:
                    c2av = VC_sb[:, cols[0] : cols[0] + 1]  # 2*c*alpha*v
                    ncav = VC_sb[:, cols[1] : cols[1] + 1]  # -c*alpha*v
                    nc.vector.tensor_scalar_mul(pa, sg[:, :NQ], c2av)
                    qg = q_grid(ci, lv)
                    nc.vector.tensor_scalar(pb, qg[:, :NQ], c2av, ncav, MUL, ADD)
                preps[key] = (pa, pb)

        emit_preps(0)  # needs SB/CB only: unblocks the first S matmuls

        C_[1] = work.tile([P, W3], F32R, tag="C1", name="C1")
        nc.vector.tensor_scalar(C_[1], Q[1], 2.0, -1.0, MUL, ADD)
        emit_preps(1)  # needs Q1/T1

        T[2] = work.tile([P, W3], F32R, tag="T2", name="T2")
        nc.gpsimd.tensor_mul(T[2], T[1], C_[1])
        C_[2] = work.tile([P, W3], F32R, tag="C2", name="C2")
        nc.vector.tensor_scalar(C_[2], Q[2], 2.0, -1.0, MUL, ADD)
        emit_preps(2)  # needs Q2/T2

        T[3] = work.tile([P, W3], F32R, tag="T3", name="T3")
        nc.gpsimd.tensor_mul(T[3], T[2], C_[2])
        C_[3] = work.tile([P, W2], F32R, tag="C3", name="C3")
        nc.vector.tensor_scalar(C_[3], Q[3][:, N:W3], 2.0, -1.0, MUL, ADD)
        emit_preps(3)  # needs Q3/T3

        T[4] = work.tile([P, W2], F32R, tag="T4", name="T4")
        nc.gpsimd.tensor_mul(T[4], T[3][:, N:W3], C_[3])
        emit_preps(4)  # needs Q4/T4

        # ---- phase 3: S^T accumulation ------------------------------------
        # ST[p, kb*NQ + i] = S[i, kb*128 + p].  Banks: {kb0,kb1} and
        # {kb2,kb3}; one accumulation group per bank.
        ST_ps = psum.tile([P, 4 * NQ], F32, tag="ST")
        order = sorted(ACTIVE, key=lambda k: k[1])
        maxlv = order[-1][1]
        # (key, kb) emission: levels < max interleave all 4 key-blocks; the
        # last level finishes bank0 (kb0/kb1) first so exp0 overlaps bank1.
        sched = []
        for key in order:
            if key[1] < maxlv:
                sched.extend((key, kb) for kb in range(4))
        lastlv = [key for key in order if key[1] == maxlv]
        sched.extend((key, kb) for kb in (0, 1) for key in lastlv)
        sched.extend((key, kb) for kb in (2, 3) for key in lastlv)
        bank_first = {}
        bank_last = {}
        for key, kb in sched:
            bank = kb // 2
            bank_first.setdefault(bank, (key, kb))
            bank_last[bank] = (key, kb)
        for key, kb in sched:
            ci, lv = key
            pa, pb = preps[key]
            sg = sin_grid(ci, lv)
            bg = CB[:, ci * N : (ci + 1) * N] if lv == 0 else q_grid(ci, lv)
            seg = slice(kb * NQ, (kb + 1) * NQ)
            kbs = slice(kb * P, (kb + 1) * P)
            # A-term: lhsT = cos-ish b-side block, rhs = weighted sin(a)
            nc.tensor.matmul(
                ST_ps[:, seg],
                bg[:, kbs],
                pa,
                start=(bank_first[kb // 2] == (key, kb)),
                stop=False,
            )
            # B-term: lhsT = sin b-side block, rhs = weighted cos(a)
            nc.tensor.matmul(
                ST_ps[:, seg],
                sg[:, kbs],
                pb,
                start=False,
                stop=(bank_last[kb // 2] == (key, kb)),
            )

        # ---- phase 4: softmax (transposed) + C ----------------------------
        # no max-subtraction: |S| <= sum(v) ~ 62 keeps exp in f32 range.
        # One exp per ST bank so the first overlaps the last matmuls.
        E_sb = work.tile([P, 4 * NQ], BF16, tag="E")
        nc.scalar.activation(E_sb[:, 0:N], ST_ps[:, 0:N], Exp)
        nc.scalar.activation(E_sb[:, N : 2 * N], ST_ps[:, N : 2 * N], Exp)

        # row sums rsum_i = sum_j E[j, i] via ones-matmuls; then 1/rsum
        rrec = []
        rs_ps = psum.tile([P, 2], F32, tag="rs")
        for h in range(2):
            for kb in range(4):
                nc.tensor.matmul(
                    rs_ps[:, h : h + 1],
                    E_sb[:, kb * NQ + h * P : kb * NQ + (h + 1) * P],
                    ones16,
                    start=(h == 0 and kb == 0),
                    stop=(h == 1 and kb == 3),
                )
            rr = small.tile([P, 1], F32, tag=f"rr{h}", name=f"rr{h}")
            nc.vector.reciprocal(rr, rs_ps[:, h : h + 1])
            rrec.append(rr)

        cp_ps = psum.tile([P, 2 * P], F32, tag="cp")
        for h in range(2):
            cph = cp_ps[:, h * P : (h + 1) * P]
            for kb in range(4):
                nc.tensor.matmul(
                    cph,
                    E_sb[:, kb * NQ + h * P : kb * NQ + (h + 1) * P],
                    Bk16[:, kb * P : (kb + 1) * P],
                    start=(h == 0 and kb == 0),
                    stop=(h == 1 and kb == 3),
                )
            c_sb = work.tile([P, P], F32, tag=f"c{h}", name=f"c{h}")
            nc.vector.tensor_scalar_mul(c_sb, cph, rrec[h])
            eng = nc.sync if h == 0 else nc.gpsimd
            eng.dma_start(out=out[h * P : (h + 1) * P, :], in_=c_sb)

    nc.compile()
    return nc


def kernel(B, W, v):
    global _program, LAST_RESULT
    B = np.ascontiguousarray(np.asarray(B, dtype=np.float32))
    W = np.ascontiguousarray(np.asarray(W, dtype=np.float32))
    v = np.asarray(v, dtype=np.float32).reshape(P)

    if _program is None:
        _program = _build_program()
    nc = _program

    WTS = np.concatenate(
        [np.float32(w0) * np.ascontiguousarray(W.T) for w0 in BASES], axis=1
    ).astype(np.float32)

    VC = np.zeros((P, VC_NCOL), dtype=np.float32)
    for key in ACTIVE:
        ci, lv = key
        cols = VC_COLS[key]
        c = COEF[key]
        if lv == 0:
            VC[:, cols[0]] = np.float32(c) * v
        else:
            alpha = float(2**lv)
            VC[:, cols[0]] = np.float32(2.0 * c * alpha) * v
            VC[:, cols[1]] = np.float32(-c * alpha) * v

    in_maps = []
    for cidx in range(NCORES):
        b = cidx // 2
        q0 = (cidx % 2) * NQ
        Bp = np.ascontiguousarray(np.roll(B[b], -q0, axis=0))
        wv = np.concatenate([WTS, VC], axis=1).astype(np.float32)
        # IN2[p, kb*128 + d] = Bp[kb*128 + p, d]  (key rows on partitions)
        in2 = np.ascontiguousarray(
            Bp.reshape(4, P, P).transpose(1, 0, 2).reshape(P, N)
        )
        in_maps.append(
            {
                "BkTd": np.ascontiguousarray(Bp.T),
                "WVd": np.ascontiguousarray(wv),
                "IN2": in2,
            }
        )

    # retry a couple of times: the axon/PJRT execute path occasionally hits
    # transient INTERNAL errors that succeed on re-run
    res = None
    for attempt in range(3):
        try:
            res = run_bass_kernel_spmd(
                nc, in_maps, core_ids=list(range(NCORES)), trace=TRACE
            )
            break
        except Exception:
            if attempt == 2:
                raise
            time.sleep(2.0)
    LAST_RESULT = res

    C = np.empty((NB, N, P), dtype=np.float32)
    for cidx in range(NCORES):
        b = cidx // 2
        q0 = (cidx % 2) * NQ
        C[b, q0 : q0 + NQ] = res.results[cidx]["out"]
    return C
